# revision 1
# baseline (speedup 1.0000x reference)
"""Chamfer distance loss kernel for Trainium2 (8 NeuronCores, SPMD).

Problem: B=4 batches, N=M=8192 points, D=3.
  loss = sum_b [ sum_i min_j ||c1_i - c2_j||^2 + sum_j min_i ||c2_j - c1_i||^2 ]

Design (all-DVE, shared distance matrix):
  Core c = (batch b = c//2, row-half h = c%2) computes the pairwise
  g_ij = |b_j|^2 - 2 a_i.b_j for its 4096 cloud1 rows x all 8192 cloud2
  points, and extracts BOTH chamfer directions from the same passes.
  Per 128-row tile (5 vector-engine instructions, each [128, 8192]):
    stt: u = (xb_rep * sx) + sqb_rep     (sx = -2*xa, per-partition scalar)
    stt: u = (yb_rep * sy) + u
    stt: u = (zb_rep * sz) + u           -> u = g_ij
    red: mins[:, t] = min_j u            (c1->c2; |a|^2 added on host)
    stt: macc = min(u + |a_i|^2, macc)   (running c2->c1 column mins;
                                          tile 0 uses op1=bypass as init)
  Host: c1->c2 = sum(mins) + sum(|a|^2); c2->c1 = elementwise min of the
  two half maccs, then min over the 128 partitions, then sum over j.

Why all-DVE: in this environment ACT instructions cost ~107us each at
[128,8192] and PE matmuls ~130us each, while DVE ops stream at
~1 elem/lane/cycle with negligible fixed cost.  scalar_tensor_tensor
(native InstTensorScalarPtr) fuses the per-partition scale multiply and
the tensor add/min into one pass, so the whole kernel is 161 DVE
instructions per core with no cross-engine semaphores in the body.
B-side coords+|b|^2 are replicated across partitions once via a single
stride-0 partition_broadcast DMA.

Toolchain notes: walrus accepts at most ONE sync-wait command per
instruction (none on custom ISA ops); _split_waits() hoists extras into
standalone event-semaphore instructions.  _strip_self_waits() removes
same-engine waits (guaranteed by program order).
"""

import numpy as np

try:
    import concourse.bass as bass  # noqa: F401
except ImportError:  # harness may run with a bare sys.path
    import sys

    for p in ("/root/.axon_site/_ro/trn_rl_repo", "/opt/trn_rl_repo", "/opt/pypackages"):
        if p not in sys.path:
            sys.path.append(p)
    import concourse.bass as bass  # noqa: F401

import re as _re

B, N, M, D = 4, 8192, 8192, 3
NCORES = 8
PT = 128          # rows per tile (SBUF partitions)
HT = N // 2       # 4096 cloud1 rows per core
RT = HT // PT     # 32 row tiles per core

_SELF_WAIT_RE = _re.compile(r"^(Pool|Activation|PE|DVE|SP)(_sequencer)?_\d+$")


def mybir_mod():
    from concourse import mybir

    return mybir


def _strip_self_waits(nc):
    """Remove semaphore waits where an instruction waits on its OWN engine's
    proc semaphore (engines execute in order with in-order data completion,
    so these are redundant and sem waits are expensive here)."""
    for f in nc.m.functions:
        for bb in f.blocks:
            for ins in bb.instructions:
                si = ins.sync_info
                if not si or not si.on_wait:
                    continue
                eng = str(ins.engine.value) if hasattr(ins.engine, "value") else str(ins.engine)
                kept = []
                for w in si.on_wait:
                    m = _SELF_WAIT_RE.match(w.ant_name or "")
                    if m and m.group(1) == eng:
                        continue
                    kept.append(w)
                if len(kept) != len(si.on_wait):
                    ins.sync_info = mybir_mod().SyncInfo(
                        on_wait=kept, on_update=list(si.on_update)
                    )
    return nc


def _split_waits(nc, max_waits=1):
    """Walrus accepts at most one sync-wait command per instruction (and none
    on custom bass_isa ops); hoist extras into standalone event-semaphore
    instructions right before the owner (same engine, program order)."""
    from concourse import mybir

    for f in nc.m.functions:
        for bb in f.blocks:
            new_insts = []
            for ins in bb.instructions:
                si = ins.sync_info
                waits = list(si.on_wait) if si and si.on_wait else []
                lim = 0 if "bass_isa" in type(ins).__module__ else max_waits
                if len(waits) > lim:
                    extra, keep = (waits, []) if lim == 0 else (waits[:-lim], waits[-lim:])
                    for k, w in enumerate(extra):
                        ev = mybir.InstEventSemaphore(
                            name=f"{ins.name}-evw{k}", ins=[], outs=[]
                        )
                        ev.engine = ins.engine
                        ev.sync_info = mybir.SyncInfo(on_wait=[w], on_update=[])
                        new_insts.append(ev)
                    ins.sync_info = mybir.SyncInfo(
                        on_wait=keep, on_update=list(si.on_update)
                    )
                new_insts.append(ins)
            bb.instructions[:] = new_insts
    return nc


def build_nc(reps=1, post=True, macc_out=True):
    """Per-core Bass program (SPMD: same program, per-core data)."""
    import concourse.tile as tile
    from concourse import mybir

    n_b = M
    rt = RT
    nc = bass.Bass("TRN2", target_bir_lowering=False, debug=False,
                   num_devices=NCORES)
    bc_d = nc.dram_tensor("bc", [1, 4 * n_b], mybir.dt.float32,
                          kind="ExternalInput")
    ac_d = nc.dram_tensor("ac", [PT, 4 * rt], mybir.dt.float32,
                          kind="ExternalInput")
    out_d = nc.dram_tensor("out", [PT, rt], mybir.dt.float32,
                           kind="ExternalOutput")
    if macc_out:
        macc_d = nc.dram_tensor("macc", [PT, n_b], mybir.dt.float32,
                                kind="ExternalOutput")

    MUL = mybir.AluOpType.mult
    ADD = mybir.AluOpType.add
    MIN = mybir.AluOpType.min
    BYP = mybir.AluOpType.bypass

    with tile.TileContext(nc) as tc:
        with tc.tile_pool(name="rep", bufs=1) as rpool:
            rep = rpool.tile([PT, 4 * n_b], mybir.dt.float32)
            nc.sync.dma_start(rep[:], bc_d[:].partition_broadcast(PT))
            with tc.tile_pool(name="work", bufs=1) as wpool:
                ac = wpool.tile([PT, 4 * rt], mybir.dt.float32)
                nc.sync.dma_start(ac[:], ac_d[:])
                mins = wpool.tile([PT, rt], mybir.dt.float32)
                macc = wpool.tile([PT, n_b], mybir.dt.float32)
                u = wpool.tile([PT, n_b], mybir.dt.float32)
                xr = rep[:, 0:n_b]
                yr = rep[:, n_b:2 * n_b]
                zr = rep[:, 2 * n_b:3 * n_b]
                qr = rep[:, 3 * n_b:4 * n_b]

                for _ in range(reps):
                    for t in range(rt):
                        sx = ac[:, t:t + 1]
                        sy = ac[:, rt + t:rt + t + 1]
                        sz = ac[:, 2 * rt + t:2 * rt + t + 1]
                        aq = ac[:, 3 * rt + t:3 * rt + t + 1]
                        nc.vector.scalar_tensor_tensor(
                            u[:], xr, sx, qr, op0=MUL, op1=ADD)
                        nc.vector.scalar_tensor_tensor(
                            u[:], yr, sy, u[:], op0=MUL, op1=ADD)
                        nc.vector.scalar_tensor_tensor(
                            u[:], zr, sz, u[:], op0=MUL, op1=ADD)
                        nc.vector.tensor_reduce(
                            mins[:, t:t + 1], u[:],
                            axis=mybir.AxisListType.X, op=MIN)
                        if t == 0:
                            # init: macc = u + |a|^2 (bypass ignores in1; u
                            # doubles as in1 so uninitialized macc is never
                            # an input)
                            nc.vector.scalar_tensor_tensor(
                                macc[:], u[:], aq, u[:], op0=ADD, op1=BYP)
                        else:
                            nc.vector.scalar_tensor_tensor(
                                macc[:], u[:], aq, macc[:], op0=ADD, op1=MIN)
                nc.sync.dma_start(out_d[:], mins[:])
                if macc_out:
                    nc.sync.dma_start(macc_d[:], macc[:])
    if post:
        return _split_waits(_strip_self_waits(nc))
    return nc


def make_in_maps(cloud1, cloud2):
    """Core 2b+h: batch b, cloud1 row-half h, vs all of cloud2."""
    in_maps = []
    sqa_half = []
    for b in range(B):
        A = np.asarray(cloud1[b], np.float32)
        Bc = np.asarray(cloud2[b], np.float32)
        sqb = (Bc * Bc).sum(-1).astype(np.float32)
        bc = np.ascontiguousarray(
            np.concatenate([Bc[:, 0], Bc[:, 1], Bc[:, 2], sqb]
                           ).reshape(1, 4 * M).astype(np.float32))
        sqa = (A * A).sum(-1).astype(np.float32)
        for h in range(2):
            Ah = A[h * HT:(h + 1) * HT]
            sqah = sqa[h * HT:(h + 1) * HT]
            ac = np.concatenate(
                [(-2.0 * Ah[:, k]).reshape(RT, PT).T for k in range(3)]
                + [sqah.reshape(RT, PT).T], axis=1).astype(np.float32)
            in_maps.append({"bc": bc, "ac": np.ascontiguousarray(ac)})
            sqa_half.append(float((Ah.astype(np.float64) ** 2).sum()))
    return in_maps, sqa_half


_NC_CACHE = {}


def kernel(cloud1, cloud2):
    from concourse.bass_utils import run_bass_kernel_spmd

    cloud1 = np.asarray(cloud1, np.float32)
    cloud2 = np.asarray(cloud2, np.float32)
    assert cloud1.shape == (B, N, D) and cloud2.shape == (B, M, D)

    if "nc" not in _NC_CACHE:
        _NC_CACHE["nc"] = build_nc()
    nc = _NC_CACHE["nc"]

    in_maps, sqa_half = make_in_maps(cloud1, cloud2)
    results = run_bass_kernel_spmd(nc, in_maps, list(range(NCORES))).results
    total = 0.0
    for b in range(B):
        r0, r1 = results[2 * b], results[2 * b + 1]
        total += float(r0["out"].astype(np.float64).sum()) + sqa_half[2 * b]
        total += float(r1["out"].astype(np.float64).sum()) + sqa_half[2 * b + 1]
        mc = np.minimum(r0["macc"], r1["macc"]).min(axis=0)
        total += float(mc.astype(np.float64).sum())
    return np.array(total, dtype=np.float32)

